# revision 26
# baseline (speedup 1.0000x reference)
"""Trainium2 Bass kernel for: x + s -> LayerNorm(W) -> 2x2x2 avgpool -> exact GELU.

Input  x: (32, 32, 16, 32, 64) f32, sum_weight (1,), gamma (64,), beta (64,)
Output:   (32, 32, 8, 16, 32) f32

Math:
  v = x + s; LN over last dim W: stats are shift-invariant => sum_weight
  cancels exactly.
  ln = (x - mu) * rho * gamma + beta,  rho = rsqrt(var + eps)
  pooled[q, w'] = (1/8) [ S - mq[q]*gw[w'] + 4*(beta_e+beta_o)[w'] ]
    S  = sum_{r in quad} rho_r * (ga*x[r,2w'] + go*x[r,2w'+1])
    mq = sum_{r in quad} (64*mu_r) * rho_r,  gw = (ga + go)/64
  out = Gelu(pooled)

Hardware facts this schedule is built around (measured on trn2):
  - DVE f32 ~1.04 ns/elem; fp16 with ALL operands 2-byte unit-inner-stride
    runs 2x.  Strided/broadcast reads are full speed WHEN GPSIMD IS IDLE.
  - DVE tensor_tensor ops co-running with GPSIMD inflate ~2.5x (SBUF
    contention); tensor_reduce ops are immune.
  - GPSIMD TT is ~1.7-2.0 ns/elem and pattern-agnostic, but fp16 reads are
    slow (~3 ns/elem).  => GPSIMD only runs f32 pair-sums (ps), scheduled
    to co-run with DVE's contention-immune reduces, plus tiny mq sums.
  - ACT is ~0.97 ns/elem, unary only: squares (fp16 parity-deinterleaved
    out), sqrt, exact GELU.

Pipeline per chunk (CHUNK = 8192 elems = 4 d-planes = 128 LN rows):
  DMA in -> ACT sq -> DVE psq + reduces (r2, r1-on-GP-ps) -> smalls (rstd,
  mr) -> DVE xr (fp16, parity-deinterleaved) -> DVE d-pool/h-pool (fp16 2x)
  -> per-chunk tail: gamma-combine + mean-correction + beta (fp16 2x) ->
  ACT GELU -> DMA out.

Layout: data-parallel over batch N (4 per core x 8 cores). Partition dim =
128 (n, c) pairs; free dim = (d, h, w).
"""

import numpy as np

import concourse.bacc as bacc
import concourse.bass as bass
import concourse.tile as tile
from concourse import mybir
from concourse.bass_utils import run_bass_kernel_spmd

P = 128
N, C, D, H, W = 32, 32, 16, 32, 64
NCORES = 8
NPER = N // NCORES
EPS = 1e-5
F32 = mybir.dt.float32
F16 = mybir.dt.float16

CHUNK = 4 * H * W          # 8192 elems / partition, 128 rows of 64
NCHUNK = D // 4            # 4
ROWS = 128                 # rows per chunk
ALU = mybir.AluOpType


def _bcast(ap, shape):
    """Broadcast [P, n] AP to shape (P, ..., n) with stride-0 middle dims."""
    while len(ap.shape) < len(shape):
        ap = ap.unsqueeze(1)
    return ap.to_broadcast(shape)


def _kernel_body(ctx, tc: tile.TileContext, out_ap: bass.AP, xs: bass.AP,
                 cons: bass.AP):
    nc = tc.nc

    singles = ctx.enter_context(tc.tile_pool(name="singles", bufs=1))
    xpool = ctx.enter_context(tc.tile_pool(name="xpool", bufs=2))
    sqpool = ctx.enter_context(tc.tile_pool(name="sqpool", bufs=1))
    pspool = ctx.enter_context(tc.tile_pool(name="pspool", bufs=1))
    xrpool = ctx.enter_context(tc.tile_pool(name="xrpool", bufs=1))
    xdpool = ctx.enter_context(tc.tile_pool(name="xdpool", bufs=1))
    smpool = ctx.enter_context(tc.tile_pool(name="smpool", bufs=2))
    tailpool = ctx.enter_context(tc.tile_pool(name="tailpool", bufs=2))

    # --- first chunk DMA before constants (cuts startup latency) ---
    xsf = xs.rearrange("p d h w -> p (d h w)")
    xc0 = xpool.tile([P, CHUNK], F32, tag="xc", name="xc0")
    half = CHUNK // 2
    for s in range(2):
        nc.sync.dma_start(out=xc0[:, s * half:(s + 1) * half],
                          in_=xsf[:, s * half:(s + 1) * half])

    # --- constants ---
    ga_t = singles.tile([P, 32], F32)
    go_t = singles.tile([P, 32], F32)
    gw_t = singles.tile([P, 32], F32)
    bw_t = singles.tile([P, 32], F32)
    for r, t in enumerate((ga_t, go_t, gw_t, bw_t)):
        nc.sync.dma_start(out=t[:], in_=cons[r:r + 1, :].to_broadcast((P, 32)))
    ga16_t = singles.tile([P, 32], F16)
    nc.vector.tensor_scalar_mul(out=ga16_t[:], in0=ga_t[:], scalar1=1.0)
    go16_t = singles.tile([P, 32], F16)
    nc.vector.tensor_scalar_mul(out=go16_t[:], in0=go_t[:], scalar1=1.0)
    bw16_t = singles.tile([P, 32], F16)
    nc.vector.tensor_scalar_mul(out=bw16_t[:], in0=bw_t[:], scalar1=1.0)
    eps_t = singles.tile([P, 1], F32)
    nc.vector.memset(eps_t[:], EPS)

    outf = out_ap.rearrange("p d h w -> p (d h w)")  # [P, 4096]

    # --- persistent staging (per chunk) ---
    rstd_c = [singles.tile([P, ROWS], F32, name=f"rstd{i}")
              for i in range(NCHUNK)]
    r1_c = [singles.tile([P, ROWS], F32, name=f"r1v{i}")
            for i in range(NCHUNK)]
    r2_c = [singles.tile([P, ROWS], F32, name=f"r2v{i}")
            for i in range(NCHUNK)]
    mr_c = [singles.tile([P, ROWS], F32, name=f"mr{i}")
            for i in range(NCHUNK)]

    def dma_in(k):
        xc = xpool.tile([P, CHUNK], F32, tag="xc")
        nc.sync.dma_start(out=xc[:], in_=xsf[:, k * CHUNK:(k + 1) * CHUNK])
        return xc

    def stats(k, xc):
        """ACT square (parity-outer fp16) + DVE psq + row reduces.

        r1 comes from a GPSIMD f32 pair-sum; both reduces are
        SBUF-contention-immune so they co-run with the GP op."""
        # x viewed as [P, parity, row, w'] (parity OUTER -> contiguous halves)
        x4o = xc[:].rearrange("p (r v t) -> p t r v", v=32, t=2)
        sq4 = sqpool.tile([P, 2, ROWS, 32], F16, tag="sq")
        nc.scalar.activation(sq4[:], x4o,
                             mybir.ActivationFunctionType.Square)
        psq = pspool.tile([P, ROWS, 32], F16, tag="psq")
        nc.vector.tensor_tensor(out=psq[:], in0=sq4[:, 0, :, :],
                                in1=sq4[:, 1, :, :], op=ALU.add)
        nc.vector.tensor_reduce(out=r2_c[k][:], in_=psq[:],
                                axis=mybir.AxisListType.X, op=ALU.add)
        x4 = xc[:].rearrange("p (r v t) -> p r t v", v=32, t=2)
        ps = pspool.tile([P, ROWS, 32], F32, tag="ps")
        nc.gpsimd.tensor_tensor(out=ps[:], in0=x4[:, :, 0, :],
                                in1=x4[:, :, 1, :], op=ALU.add)
        nc.vector.tensor_reduce(out=r1_c[k][:], in_=ps[:],
                                axis=mybir.AxisListType.X, op=ALU.add)

    def smalls(k):
        """Stats recombination for a chunk (128 rows): rstd, mr = 64*mu*rstd."""
        r1v, r2v = r1_c[k][:], r2_c[k][:]
        sqm = smpool.tile([P, ROWS], F32, tag="sqm")
        nc.vector.tensor_tensor(out=sqm[:], in0=r1v, in1=r1v, op=ALU.mult)
        # v64 = r2 - sqm/64  (= 64 * var)
        v64 = smpool.tile([P, ROWS], F32, tag="v64")
        nc.vector.scalar_tensor_tensor(out=v64[:], in0=sqm[:],
                                       scalar=-1.0 / W, in1=r2v,
                                       op0=ALU.mult, op1=ALU.add)
        sd = smpool.tile([P, ROWS], F32, tag="sd")
        nc.scalar.activation(sd[:], v64[:],
                             mybir.ActivationFunctionType.Sqrt,
                             bias=eps_t[:], scale=1.0 / W)
        nc.vector.reciprocal(out=rstd_c[k][:], in_=sd[:])
        nc.vector.tensor_tensor(out=mr_c[k][:], in0=r1v, in1=rstd_c[k][:],
                                op=ALU.mult)

    def xr_op(k, xc):
        """xr = x*rstd (fp16, parity-deinterleaved out) on DVE."""
        x4 = xc[:].rearrange("p (r v t) -> p r t v", v=32, t=2)
        xr = xrpool.tile([P, ROWS, 2, 32], F16, tag="xr")
        rb = rstd_c[k][:].unsqueeze(2).unsqueeze(3).to_broadcast(
            (P, ROWS, 2, 32))
        nc.vector.tensor_tensor(out=xr[:], in0=x4, in1=rb, op=ALU.mult)
        return xr

    def pools_tail(k, xr):
        """d-pool + h-pool (DVE fp16 2x), then the full tail for this chunk:
        gamma combine + mean correction + beta + GELU + out DMA."""
        # d-pool: rows (dsub0+dsub1) and (dsub2+dsub3): [P, 2, 2, 2048]
        xd = xdpool.tile([P, 2, 2048], F16, tag="xd")
        xr4 = xr[:].rearrange("p r t v -> p (r t v)").rearrange(
            "p (a s f) -> p a s f", a=2, s=2)
        nc.vector.tensor_tensor(out=xd[:], in0=xr4[:, :, 0, :],
                                in1=xr4[:, :, 1, :], op=ALU.add)
        # h-pool: [P, 2, 16, 2, 64] -> xh [P, 2, 16, 64]  (64 = t,v)
        xd5 = xd[:].rearrange("p a (h s f) -> p a h s f", s=2, f=64)
        xh = xdpool.tile([P, 2, 16, 64], F16, tag="xh")
        nc.vector.tensor_tensor(out=xh[:], in0=xd5[:, :, :, 0, :],
                                in1=xd5[:, :, :, 1, :], op=ALU.add)

        # mq = sum_quad (64*mu*rstd): mr rows = (dpair, dsub, h16, hpar)
        mr5 = mr_c[k][:].rearrange("p (a d q t) -> p a d q t", a=2, d=2, t=2)
        mq1 = tailpool.tile([P, 2, 2, 16], F32, tag="mq1")
        nc.gpsimd.tensor_tensor(out=mq1[:], in0=mr5[:, :, :, :, 0],
                                in1=mr5[:, :, :, :, 1], op=ALU.add)
        mq = tailpool.tile([P, 2, 16], F32, tag="mq")
        nc.gpsimd.tensor_tensor(out=mq[:], in0=mq1[:, :, 0, :],
                                in1=mq1[:, :, 1, :], op=ALU.add)

        sh3 = (P, 32, 32)
        xhf = xh[:].rearrange("p a h (t v) -> p (a h) t v", t=2)
        t1 = tailpool.tile([P, 32, 32], F16, tag="t1")
        nc.vector.tensor_tensor(out=t1[:], in0=xhf[:, :, 0, :],
                                in1=_bcast(ga16_t[:], sh3), op=ALU.mult)
        t2 = tailpool.tile([P, 32, 32], F16, tag="t2")
        nc.vector.tensor_tensor(out=t2[:], in0=xhf[:, :, 1, :],
                                in1=_bcast(go16_t[:], sh3), op=ALU.mult)
        s_t = tailpool.tile([P, 32, 32], F16, tag="s")
        nc.vector.tensor_tensor(out=s_t[:], in0=t1[:], in1=t2[:], op=ALU.add)
        corr = tailpool.tile([P, 32, 32], F16, tag="corr")
        mqb = mq[:].rearrange("p a h -> p (a h)").unsqueeze(2).to_broadcast(
            sh3)
        nc.vector.tensor_tensor(out=corr[:], in0=mqb,
                                in1=_bcast(gw_t[:], sh3), op=ALU.mult)
        pre = tailpool.tile([P, 32, 32], F16, tag="pre")
        nc.vector.tensor_tensor(out=pre[:], in0=s_t[:], in1=corr[:],
                                op=ALU.subtract)
        pre2 = tailpool.tile([P, 32, 32], F16, tag="pre2")
        nc.vector.tensor_tensor(out=pre2[:], in0=pre[:],
                                in1=_bcast(bw16_t[:], sh3), op=ALU.add)
        res = tailpool.tile([P, 1024], F32, tag="res")
        nc.scalar.activation(res[:], pre2[:].rearrange("p a b -> p (a b)"),
                             mybir.ActivationFunctionType.Gelu, scale=0.125)
        nc.sync.dma_start(out=outf[:, k * 1024:(k + 1) * 1024], in_=res[:])

    # ---- schedule: chunk cadence; GP ps(k+1) co-runs with DVE reduces ----
    xc_t = [None] * NCHUNK
    xc_t[0] = xc0
    xc_t[1] = dma_in(1)
    stats(0, xc_t[0])
    for k in range(NCHUNK):
        smalls(k)
        xr_k = xr_op(k, xc_t[k])
        if k + 1 < NCHUNK:
            if k + 2 < NCHUNK:
                xc_t[k + 2] = dma_in(k + 2)
            stats(k + 1, xc_t[k + 1])
        pools_tail(k, xr_k)


_CACHE: dict = {}


def _get_compiled():
    if "nc" not in _CACHE:
        nc = bacc.Bacc("TRN2", target_bir_lowering=False, debug=False)
        xs = nc.dram_tensor("xs", [P, D, H, W], F32, kind="ExternalInput").ap()
        cons = nc.dram_tensor("cons", [4, 32], F32, kind="ExternalInput").ap()
        out = nc.dram_tensor(
            "out", [P, D // 2, H // 2, W // 2], F32, kind="ExternalOutput"
        ).ap()
        from contextlib import ExitStack

        with tile.TileContext(nc) as tc, ExitStack() as ctx:
            _kernel_body(ctx, tc, out, xs, cons)
        nc.compile()
        _CACHE["nc"] = nc
    return _CACHE["nc"]


def _make_cons(gamma: np.ndarray, beta: np.ndarray) -> np.ndarray:
    ga = gamma[0::2].astype(np.float64)
    go = gamma[1::2].astype(np.float64)
    # mr carries 64*mu*rstd -> fold the 1/64 into gw
    gw = (ga + go) / 64.0
    bw = 4.0 * (beta[0::2].astype(np.float64) + beta[1::2].astype(np.float64))
    return np.stack([ga, go, gw, bw]).astype(np.float32)


def kernel(x, sum_weight, gamma, beta, trace=False):
    del sum_weight  # cancels exactly in LayerNorm (shift invariance)
    nc = _get_compiled()
    x = np.ascontiguousarray(np.asarray(x), dtype=np.float32)
    cons = _make_cons(np.asarray(gamma), np.asarray(beta))
    in_maps = []
    for core in range(NCORES):
        shard = x[core * NPER:(core + 1) * NPER].reshape(P, D, H, W)
        in_maps.append({"xs": shard, "cons": cons})
    res = run_bass_kernel_spmd(nc, in_maps, core_ids=list(range(NCORES)),
                               trace=trace)
    out = np.concatenate(
        [
            res.results[i]["out"].reshape(NPER, C, D // 2, H // 2, W // 2)
            for i in range(NCORES)
        ],
        axis=0,
    )
    if trace:
        return out, res
    return out


if __name__ == "__main__":
    rng = np.random.default_rng(0)
    x = rng.standard_normal((N, C, D, H, W), dtype=np.float32)
    sw = rng.standard_normal((1,)).astype(np.float32)
    gamma = rng.random((W,), dtype=np.float32)
    beta = rng.standard_normal((W,)).astype(np.float32)
    y = kernel(x, sw, gamma, beta)
    print(y.shape, y.dtype)
